# revision 1
# baseline (speedup 1.0000x reference)
"""Trainium2 Bass kernel for nn_AttentionITBlock — v2 (fp16, single basis load).

Contract: kernel(**inputs) takes FULL unsharded inputs, returns FULL output
(B, C, H, W) float32. Data-parallel over batch B=8, one batch per core.

Key ideas vs v1:
  - All large tensors staged to device as fp16 (basis 2x smaller, x 2x, out 2x).
  - Basis loaded ONCE into SBUF (resident 162KB/partition, Re|Im concat per
    128-hw-chunk); inverse-transform operands transposed on-chip on the PE
    (the xbar DMA-transpose path corrupts data on this runtime - avoid).
  - x streamed and transposed on-chip for the forward; re-streamed for the
    shortcut.
  - Complex packing: Q/K/V projections emit [re;im] per head so logits
    (Re(q k*)) and attn@v are single matmuls over K=64; softmax row-sums ride
    as a 65th column (ones column appended to V^T).
  - alpha-scale + mixer + LN mean-centering folded into one host-prepared
    matrix applied in spectral space (576 tokens instead of 9216 pixels).
  - LN variance via per-column ones-matmuls; rsqrt via fp32 bit-hack +
    2 Newton steps (DVE only, no ACT table swaps); broadcast via K=1 matmuls.
  - Emission is software-pipelined: attention interleaves the two heads of a
    pair (PE fills ACT exp latency); the inverse/LN tail emits block k's
    finish after block k+1's start so consecutive blocks overlap in the
    in-order engine queues.

PSUM bank map (8 x 2KB): A/B/C = fwd-acc, attn po, pxm ping-pong + shortcut;
D = attn po, LN var column; E(x2 ring) = proj/logit/mixer matmuls,
basis-transpose staging, rstd row; F(x2 ring) = V^T staging, softmax-sum
broadcast, rstd broadcast.
"""

import sys

sys.path.insert(0, "/opt/trn_rl_repo")

import numpy as np

import concourse.bass as bass
import concourse.mybir as mybir
import concourse.tile as tile
from concourse import bacc
from concourse.bass_utils import run_bass_kernel_spmd
from concourse.masks import make_identity

F32 = mybir.dt.float32
F16 = mybir.dt.float16
AF = mybir.ActivationFunctionType
I32 = mybir.dt.int32

B, C, H, W = 8, 128, 96, 96
HW = H * W                      # 9216
M1 = M2 = 24
S = M1 * M2                     # 576 tokens
SCAT = 2 * S                    # 1152 = re|im concatenated token axis
NH, DH = 4, 32
SCALE = 1.0 / np.sqrt(np.float32(DH))

NCH = HW // 128                 # 72 hw chunks of 128
NBLK = HW // 512                # 18 hw blocks of 512
NT = SCAT // 128                # 9 token tiles over the concatenated axis
TT = [(i * 128, min(128, S - i * 128)) for i in range((S + 127) // 128)]  # 5
MAGIC = 0x5F3759DF


def build_module(gelu=AF.Gelu_apprx_tanh):
    nc = bacc.Bacc("TRN2", target_bir_lowering=False, debug=False)

    d_x = nc.dram_tensor("x16", [C, HW], F16, kind="ExternalInput").ap()
    d_bs = nc.dram_tensor("bs16", [128, NCH, SCAT], F16, kind="ExternalInput").ap()
    d_wqkv = nc.dram_tensor("wqkv", [C, 12, C], F16, kind="ExternalInput").ap()
    d_bqkv = nc.dram_tensor("bqkv", [C, 6], F32, kind="ExternalInput").ap()
    d_wo = nc.dram_tensor("wo", [64, 8, C], F16, kind="ExternalInput").ap()
    d_bo = nc.dram_tensor("bo", [C, 2], F32, kind="ExternalInput").ap()
    d_w2t = nc.dram_tensor("w2t", [C, C], F16, kind="ExternalInput").ap()
    d_b2 = nc.dram_tensor("b2", [C], F32, kind="ExternalInput").ap()
    d_gam = nc.dram_tensor("gam", [C], F32, kind="ExternalInput").ap()
    d_bet = nc.dram_tensor("bet", [C], F32, kind="ExternalInput").ap()
    d_scw = nc.dram_tensor("scwT", [C, C], F16, kind="ExternalInput").ap()
    d_scb = nc.dram_tensor("scb", [C], F32, kind="ExternalInput").ap()
    d_out = nc.dram_tensor("out16", [C, HW], F16, kind="ExternalOutput").ap()

    with tile.TileContext(nc) as tc:
        _body(nc, tc, d_x, d_bs, d_wqkv, d_bqkv, d_wo, d_bo, d_w2t, d_b2,
              d_gam, d_bet, d_scw, d_scb, d_out, gelu)
    nc.finalize()
    return nc


def _body(nc, tc, d_x, d_bs, d_wqkv, d_bqkv, d_wo, d_bo, d_w2t, d_b2,
          d_gam, d_bet, d_scw, d_scb, d_out, gelu):
    from contextlib import ExitStack
    ctx = ExitStack()
    singles = ctx.enter_context(tc.tile_pool(name="singles", bufs=1))
    work = ctx.enter_context(tc.tile_pool(name="work", bufs=2))
    small = ctx.enter_context(tc.tile_pool(name="small", bufs=1))
    ps = ctx.enter_context(tc.tile_pool(name="ps", bufs=1, space="PSUM"))

    # ---------------- constants ----------------
    ident_f = work.tile([128, 128], F32, tag="mh", name="ident_f")
    make_identity(nc, ident_f)
    ident16 = singles.tile([128, 128], F16)
    nc.vector.tensor_copy(ident16, ident_f)
    nc.vector.memset(ident_f, 1.0)  # reuse staging buffer for all-ones
    ones16 = singles.tile([128, 128], F16, tag="ones16")
    nc.vector.tensor_copy(ones16, ident_f)

    # ---------------- small weights (scalar queue; bulk loads go on sync) ----
    wqkv = singles.tile([C, 12, C], F16)
    nc.scalar.dma_start(wqkv, d_wqkv)
    wo = singles.tile([64, 8, C], F16)
    nc.scalar.dma_start(wo, d_wo)
    w2t = singles.tile([C, C], F16, tag="w2t")
    nc.scalar.dma_start(w2t, d_w2t)
    scw = singles.tile([C, C], F16, tag="scw")
    nc.scalar.dma_start(scw, d_scw)
    bqkv = singles.tile([C, 6], F32, tag="bqkv")
    nc.scalar.dma_start(bqkv, d_bqkv)
    bo = singles.tile([C, 2], F32, tag="bo")
    nc.scalar.dma_start(bo, d_bo)

    def load_col(ap1d, nm):
        t = singles.tile([C, 1], F32, tag=nm, name=nm)
        nc.scalar.dma_start(t, ap1d[:, None])
        return t

    b2c = load_col(d_b2, "b2c")
    gam = load_col(d_gam, "gam")
    bet = load_col(d_bet, "bet")
    scb = load_col(d_scb, "scb")

    # ---------------- phase 1: forward transform ----------------
    # Interleave xT / bs piece loads with consumption: 12 pieces of 6 chunks.
    bs = singles.tile([128, NCH, SCAT], F16, tag="bs")
    NP = 18
    per = NCH // NP  # 4

    def load_piece(q):
        xb = work.tile([C, per * 128], F16, tag="xt", bufs=3)
        nc.sync.dma_start(xb, d_x[:, q * per * 128:(q + 1) * per * 128])
        nc.sync.dma_start(bs[:, q * per:(q + 1) * per, :],
                          d_bs[:, q * per:(q + 1) * per, :])
        return xb

    ph = [ps.tile([C, 384], F32, tag="ABC"[i], name=f"ph{i}", bufs=1)
          for i in range(3)]
    xb_cur = load_piece(0)
    for q in range(NP):
        xb_nxt = load_piece(q + 1) if q + 1 < NP else None
        for j in range(per):
            k = per * q + j
            pt = ps.tile([128, 128], F16, tag="EF"[k % 2], name="pt", bufs=2)
            nc.tensor.transpose(pt, xb_cur[:, 128 * j:128 * (j + 1)], ident16)
            xtk = work.tile([128, 128], F16, tag="xtk", bufs=3)
            if k % 2 == 0:
                nc.vector.tensor_copy(xtk, pt)
            else:
                nc.scalar.copy(xtk, pt)
            st = dict(start=(k == 0), stop=(k == NCH - 1))
            for i in range(3):
                nc.tensor.matmul(ph[i], xtk,
                                 bs[:, k, 384 * i:384 * (i + 1)], **st)
        xb_cur = xb_nxt

    Tcat = singles.tile([C, SCAT], F16, tag="Tcat")
    for i in range(3):
        nc.any.tensor_copy(Tcat[:, 384 * i:384 * (i + 1)], ph[i])

    # ---------------- phase 2: attention ----------------
    ots = []  # per head [65, 576] f16: rows 0:32 o_re, 32:64 o_im, 64 sums
    for p in range(2):
        PP = {}
        for i in range(3):  # q, k, v
            dst = singles.tile([C, S], F16, tag=f"pp{i}", name=f"pp{i}_{p}")
            pp = ps.tile([C, 512], F32, tag="E", name="pp", bufs=2)
            nc.tensor.matmul(pp, wqkv[:, (i * 2 + p) * 2, :], Tcat[:, 0:512],
                             start=True, stop=False)
            nc.tensor.matmul(pp, wqkv[:, (i * 2 + p) * 2 + 1, :],
                             Tcat[:, S:S + 512], start=False, stop=True)
            with nc.allow_low_precision(reason="fp16 qkv"):
                nc.vector.tensor_scalar(dst[:, 0:512], pp,
                                        bqkv[:, i * 2 + p:i * 2 + p + 1],
                                        None, mybir.AluOpType.add)
            pp64 = ps.tile([C, 64], F32, tag="F", name="pp64", bufs=2)
            nc.tensor.matmul(pp64, wqkv[:, (i * 2 + p) * 2, :], Tcat[:, 512:576],
                             start=True, stop=False)
            nc.tensor.matmul(pp64, wqkv[:, (i * 2 + p) * 2 + 1, :],
                             Tcat[:, S + 512:S + 576], start=False, stop=True)
            with nc.allow_low_precision(reason="fp16 qkv"):
                nc.vector.tensor_scalar(dst[:, 512:576], pp64,
                                        bqkv[:, i * 2 + p:i * 2 + p + 1],
                                        None, mybir.AluOpType.add)
            PP[i] = dst
        QP, KP, VP = PP[0], PP[1], PP[2]

        # two heads of the pair, interleaved so PE work fills ACT exp latency
        po = {sub: ps.tile([65, 512], F32, tag="AB"[sub], name=f"po{sub}",
                           bufs=1) for sub in range(2)}
        po64 = {sub: ps.tile([65, 64], F32, tag="CD"[sub], name=f"po64_{sub}",
                             bufs=1) for sub in range(2)}
        for ti, (to, tsz) in enumerate(TT):
            ats = {}
            for sub in range(2):
                r0 = 64 * sub
                hsl = slice(r0, r0 + 64)
                tp = (r0, 0) if sub else None
                at = work.tile([128, S], F16, tag="attnT", bufs=3)
                pl = ps.tile([128, 512], F32, tag="E", name="pl", bufs=2)
                nc.tensor.matmul(pl[:tsz], KP[hsl, to:to + tsz], QP[hsl, 0:512],
                                 start=True, stop=True, tile_position=tp)
                nc.scalar.activation(at[:tsz, 0:512], pl[:tsz], AF.Exp,
                                     scale=float(SCALE))
                pl64 = ps.tile([128, 64], F32, tag="F", name="pl64", bufs=2)
                nc.tensor.matmul(pl64[:tsz], KP[hsl, to:to + tsz],
                                 QP[hsl, 512:576],
                                 start=True, stop=True, tile_position=tp)
                nc.scalar.activation(at[:tsz, 512:576], pl64[:tsz], AF.Exp,
                                     scale=float(SCALE))
                pv = ps.tile([128, 64], F16, tag="F", name="pv", bufs=2)
                nc.tensor.transpose(pv[:tsz], VP[hsl, to:to + tsz],
                                    ident16[hsl, hsl], tile_position=tp)
                vh = work.tile([128, 65], F16, tag="vh")
                nc.any.tensor_copy(vh[:tsz, 0:64], pv[:tsz])
                nc.vector.tensor_copy(vh[:tsz, 64:65], ones16[:tsz, 0:1])
                ats[sub] = (at, vh)
            for sub in range(2):
                at, vh = ats[sub]
                nc.tensor.matmul(po[sub], vh[:tsz], at[:tsz, 0:512],
                                 start=(ti == 0), stop=(ti == len(TT) - 1))
                nc.tensor.matmul(po64[sub], vh[:tsz], at[:tsz, 512:576],
                                 start=(ti == 0), stop=(ti == len(TT) - 1))
        for sub in range(2):
            h = 2 * p + sub
            ot = singles.tile([65, S], F16, tag=f"ot{h}", name=f"ot{h}")
            nc.vector.tensor_copy(ot[:, 0:512], po[sub])
            nc.scalar.copy(ot[:, 512:576], po64[sub])
            rcp = work.tile([64, S], F16, tag="attnT", name="rcp", bufs=3)
            rb = ps.tile([64, 512], F32, tag="F", name="rb", bufs=2)
            nc.tensor.matmul(rb, ones16[64:65, 0:64], ot[64:65, 0:512],
                             start=True, stop=True, tile_position=(64, 0))
            with nc.allow_low_precision(reason="softmax 1/sum in fp16"):
                nc.vector.reciprocal(rcp[:, 0:512], rb)
            rb64 = ps.tile([64, 64], F32, tag="F", name="rb64", bufs=2)
            nc.tensor.matmul(rb64, ones16[64:65, 0:64], ot[64:65, 512:576],
                             start=True, stop=True, tile_position=(64, 0))
            with nc.allow_low_precision(reason="softmax 1/sum in fp16"):
                nc.vector.reciprocal(rcp[:, 512:576], rb64)
            with nc.allow_low_precision(reason="fp16 softmax weights"):
                nc.vector.tensor_mul(ot[0:64, :], ot[0:64, :], rcp)
            ots.append(ot)

    btq = {}

    def blk_transposes(kb):
        bts = []
        for hb in range(2):
            bt = work.tile([128, NT, 256], F16, tag="bt")
            for j in range(2):
                k = 4 * kb + 2 * hb + j
                csl = slice(128 * j, 128 * (j + 1))
                for g, (t0, gsz) in enumerate(((0, 5), (5, 4))):
                    pbt = ps.tile([128, 5, 128], F16, tag="EF"[(2 * j + g) % 2],
                                  name="pbt", bufs=2)
                    for u in range(gsz):
                        t = t0 + u
                        nc.tensor.transpose(pbt[:, u, :],
                                            bs[:, k, 128 * t:128 * (t + 1)],
                                            ident16)
                    if g % 2 == 0:
                        nc.vector.tensor_copy(bt[:, t0:t0 + gsz, csl],
                                              pbt[:, 0:gsz, :])
                    else:
                        nc.scalar.copy(bt[:, t0:t0 + gsz, csl],
                                       pbt[:, 0:gsz, :])
            bts.append(bt)
        btq[kb] = bts

    blk_transposes(0)
    # output projection -> Ycat [c, re|im] f16 (+bias); shares Tcat's buffer
    Ycat = singles.tile([C, SCAT], F16, tag="Tcat", name="Ycat")
    for part in range(2):
        py = ps.tile([C, 512], F32, tag="E", name="py", bufs=2)
        for h in range(NH):
            nc.tensor.matmul(py, wo[:, part * 4 + h, :], ots[h][0:64, 0:512],
                             start=(h == 0), stop=(h == NH - 1))
        with nc.allow_low_precision(reason="fp16 Y"):
            nc.vector.tensor_scalar(Ycat[:, S * part:S * part + 512], py,
                                    bo[:, part:part + 1], None,
                                    mybir.AluOpType.add)
        py64 = ps.tile([C, 64], F32, tag="F", name="py64", bufs=2)
        for h in range(NH):
            nc.tensor.matmul(py64, wo[:, part * 4 + h, :], ots[h][0:64, 512:576],
                             start=(h == 0), stop=(h == NH - 1))
        with nc.allow_low_precision(reason="fp16 Y"):
            nc.vector.tensor_scalar(Ycat[:, S * part + 512:S * part + 576], py64,
                                    bo[:, part:part + 1], None,
                                    mybir.AluOpType.add)

    # spectral mixer: yh[s', o] = sum_c Ycat[c, s'] * W2T[c, o]
    yh = singles.tile([128, NT, C], F16, tag="yh")
    for t in range(NT):
        pyh = ps.tile([128, C], F32, tag="E", name="pyh", bufs=2)
        nc.tensor.matmul(pyh, Ycat[:, 128 * t:128 * (t + 1)], w2t,
                         start=True, stop=True)
        nc.any.tensor_copy(yh[:, t, :], pyh)

    # ---------------- phase 3: inverse transform + LN tail ----------------
    # Software pipeline: emit finish(k-1) between start(k) and start(k+1);
    # basis transposes for block 0 are hoisted into the attention tail.
    state = {}
    def blk_start(kb):
        bsl = slice(kb * 512, (kb + 1) * 512)
        pxm = ps.tile([C, 512], F32, tag="AB"[kb % 2], name="pxm", bufs=1)
        if kb not in btq:
            blk_transposes(kb)
        bts = btq.pop(kb)
        for hb in range(2):
            for t in range(NT):
                nc.tensor.matmul(pxm[:, 256 * hb:256 * (hb + 1)], yh[:, t, :],
                                 bts[hb][:, t, :],
                                 start=(t == 0), stop=(t == NT - 1))
        mh = work.tile([C, 512], F16, tag="mh")
        nc.scalar.activation(mh, pxm, AF.Identity, bias=b2c, scale=1.0)
        xblk = work.tile([C, 512], F16, tag="xa")
        nc.sync.dma_start(xblk, d_x[:, bsl])
        # square for the variance (variance matmul itself happens in finish)
        sq = work.tile([C, 512], F16, tag="sq", bufs=2)
        with nc.allow_low_precision(reason="fp16 m^2 for variance"):
            nc.gpsimd.tensor_mul(sq, mh, mh)
        state[kb] = (mh, xblk, sq, bsl)

    def blk_finish(kb):
        mh, xblk, sq, bsl = state.pop(kb)
        psc = ps.tile([C, 512], F32, tag="C", name="psc", bufs=1)
        nc.tensor.matmul(psc, scw, xblk, start=True, stop=True)
        # variance column: vcol[p, j] = sum_c sq[c, 128j+p]
        vcol = ps.tile([128, 4], F32, tag="D", name="vcol", bufs=1)
        for j in range(4):
            nc.tensor.matmul(vcol[:, j:j + 1], sq[:, 128 * j:128 * (j + 1)],
                             ones16[:, 0:1], start=True, stop=True)
        v4 = small.tile([128, 4], F32, tag="v4", bufs=1)
        nc.vector.tensor_scalar(v4, vcol, 1.0 / C, 1e-5, mybir.AluOpType.mult,
                                mybir.AluOpType.add)
        ysh = small.tile([128, 4], I32, tag="ysh", bufs=1)
        nc.vector.tensor_scalar(ysh, v4.bitcast(I32), 1, None,
                                mybir.AluOpType.logical_shift_right)
        y0 = small.tile([128, 4], I32, tag="y0", bufs=1)
        nc.vector.tensor_scalar(y0, ysh, -1, MAGIC, mybir.AluOpType.mult,
                                mybir.AluOpType.add)
        y = y0.bitcast(F32)
        for it in range(2):
            t1 = small.tile([128, 4], F32, tag=f"nt1_{it}", bufs=1)
            nc.vector.tensor_mul(t1, v4, y)
            t2 = small.tile([128, 4], F32, tag=f"nt2_{it}", bufs=1)
            nc.vector.tensor_mul(t2, t1, y)
            t3 = small.tile([128, 4], F32, tag=f"nt3_{it}", bufs=1)
            nc.vector.tensor_scalar(t3, t2, -0.5, 1.5, mybir.AluOpType.mult,
                                    mybir.AluOpType.add)
            yn = small.tile([128, 4], F32, tag=f"nyn_{it}", bufs=1)
            nc.vector.tensor_mul(yn, y, t3)
            y = yn
        y16 = small.tile([128, 4], F16, tag="y16", bufs=1)
        with nc.allow_low_precision(reason="fp16 rstd"):
            nc.vector.tensor_copy(y16, y)
        prr = ps.tile([1, 512], F16, tag="E", name="prr", bufs=2)
        for j in range(4):
            nc.tensor.transpose(prr[:, 128 * j:128 * (j + 1)], y16[:, j:j + 1],
                                ident16)
        rrow = work.tile([1, 512], F16, tag="ob", name="rrow")
        nc.vector.tensor_copy(rrow, prr)
        rbp = ps.tile([C, 512], F32, tag="F", name="rbp", bufs=2)
        for j in range(4):
            nc.tensor.matmul(rbp[:, 128 * j:128 * (j + 1)], ones16[0:1, :],
                             rrow[:, 128 * j:128 * (j + 1)],
                             start=True, stop=True)
        ln = work.tile([C, 512], F16, tag="ln", bufs=1)
        with nc.allow_low_precision(reason="fp16 normalized activations"):
            nc.vector.tensor_mul(ln, mh, rbp)
        g1 = work.tile([C, 512], F16, tag="g1", bufs=1)
        nc.scalar.activation(g1, ln, gelu, bias=bet, scale=gam)
        g2 = work.tile([C, 512], F16, tag="ln", name="g2", bufs=1)
        with nc.allow_low_precision(reason="fp16 pre-activation"):
            nc.vector.tensor_add(g2, g1, psc)
        ob = work.tile([C, 512], F16, tag="ob")
        nc.scalar.activation(ob, g2, gelu, bias=scb, scale=1.0)
        nc.scalar.dma_start(d_out[:, bsl], ob)

    blk_start(0)
    for kb in range(1, NBLK):
        blk_start(kb)
        blk_finish(kb - 1)
    blk_finish(NBLK - 1)

    ctx.close()


def _prep_inputs(inputs):
    """Host-side packing/precompute. Returns per-core in_maps."""
    f16 = np.float16
    f32 = np.float32
    x = np.asarray(inputs["x"], f32)
    br = np.asarray(inputs["basis_real"], f32)
    bi = np.asarray(inputs["basis_imag"], f32)
    awr = np.asarray(inputs["attn_w_r"], f32)
    awi = np.asarray(inputs["attn_w_i"], f32)
    abr = np.asarray(inputs["attn_b_r"], f32)
    abi = np.asarray(inputs["attn_b_i"], f32)
    alpha = np.asarray(inputs["alpha"], f32)
    mw = np.asarray(inputs["mixer_w"], f32)
    mb = np.asarray(inputs["mixer_b"], f32)
    gam = np.asarray(inputs["norm_gamma"], f32)
    bet = np.asarray(inputs["norm_beta"], f32)
    scw = np.asarray(inputs["shortcut_w"], f32)
    scb = np.asarray(inputs["shortcut_b"], f32)

    # qkv packed: [cin, 12, cout] with j = (i*2 + p)*2 + ab
    wqkv = np.empty((C, 12, C), f32)
    bqkv = np.empty((C, 6), f32)
    for i in range(3):
        wrT = awr[i].T  # [cin, cout]
        wiT = awi[i].T
        for p in range(2):
            h0 = slice(64 * p, 64 * p + 32)
            h1 = slice(64 * p + 32, 64 * p + 64)
            A = np.concatenate([wrT[:, h0], wiT[:, h0], wrT[:, h1], wiT[:, h1]], 1)
            Bm = np.concatenate([-wiT[:, h0], wrT[:, h0], -wiT[:, h1], wrT[:, h1]], 1)
            wqkv[:, (i * 2 + p) * 2, :] = A
            wqkv[:, (i * 2 + p) * 2 + 1, :] = Bm
            bqkv[:, i * 2 + p] = np.concatenate(
                [abr[i][h0], abi[i][h0], abr[i][h1], abi[i][h1]])
    # o-proj packed: wo [64, 8, C]; j = part*4 + h
    worT = awr[3].T
    woiT = awi[3].T
    wo = np.empty((64, 8, C), f32)
    for h in range(NH):
        hs = slice(32 * h, 32 * h + 32)
        wo[:, 0 * 4 + h, :] = np.concatenate([worT[hs], -woiT[hs]], 0)
        wo[:, 1 * 4 + h, :] = np.concatenate([woiT[hs], worT[hs]], 0)
    bo = np.stack([abr[3], abi[3]], 1)  # [C, 2]

    # mixer: fold alpha and LN mean-centering
    W1 = mw * alpha[None, :]
    W2 = W1 - W1.mean(0, keepdims=True)
    b2 = mb - mb.mean()

    shared = {
        "wqkv": wqkv.astype(f16), "bqkv": bqkv,
        "wo": wo.astype(f16), "bo": bo,
        "w2t": np.ascontiguousarray(W2.T).astype(f16), "b2": b2,
        "gam": gam, "bet": bet,
        "scwT": np.ascontiguousarray(scw.T).astype(f16), "scb": scb,
    }
    in_maps = []
    for b in range(B):
        m = dict(shared)
        xb = x[b].reshape(C, HW)
        m["x16"] = np.ascontiguousarray(xb).astype(f16)
        br3 = br[b].reshape(NCH, 128, S)
        bi3 = bi[b].reshape(NCH, 128, S)
        bsb = np.concatenate([br3, bi3], 2).transpose(1, 0, 2)
        m["bs16"] = np.ascontiguousarray(bsb).astype(f16)
        in_maps.append(m)
    return in_maps


_CACHE = {}
PROFILE = False
LAST_RESULTS = None


def _get_module():
    if "nc" not in _CACHE:
        _CACHE["nc"] = build_module()
    return _CACHE["nc"]


def kernel(**inputs):
    nc = _get_module()
    in_maps = _prep_inputs(inputs)
    global LAST_RESULTS
    res = run_bass_kernel_spmd(nc, in_maps, core_ids=list(range(B)), trace=PROFILE)
    LAST_RESULTS = res
    out = np.stack([np.asarray(res.results[b]["out16"]).astype(np.float32)
                    .reshape(C, H, W) for b in range(B)])
    return out

